# revision 25
# baseline (speedup 1.0000x reference)
"""Trainium2 Bass kernel for MultiHeadedSelfAttention (fastexp softmax).

Sharding: sequence-parallel over 8 cores. Each core computes K/V for the
full sequence and attention for its own 512-row query block; outputs are
disjoint row blocks of the final [4096, 512] result, so no collectives.

Device layout is "transposed everything": activations stored feature-major
(d on partitions) so projections and scores feed the PE contraction dim
directly. Softmax row-max is computed from a stride-STATS_SUB subsampled
[l, m] score pass (DVE reduce); the softmax ratio is invariant to a uniform
row shift, so a slightly-low max only perturbs the Schraudolph epsilon
pattern by ~eps' * delta. Scores are then recomputed transposed [m, l]
with the row max injected as an extra contraction row (ones x -mu), so the
exponent input arrives in PSUM already max-subtracted. The Schraudolph
fastexp is applied in the fp16 bit domain (ACT affine with int16 output,
or DVE tensor_scalar for a share of tiles to balance engines). PV runs in
[l, d] orientation (full 128-row PE utilization): four l-block
accumulators share one PSUM bank using a single start=True zero-region
mark. Row sums come free as a ones column appended to V; normalization
happens in [l, d] before a single f16 transpose back to attnT.
"""

import numpy as np

DIM = 512
H = 8
HD = 64
SEQ = 4096
NCORES = 8
LSP = SEQ // NCORES  # 512 query rows per core

GIST_A = 12102203.17133801
GIST_B = 1064986823.0


def _q_rsqrt(x):
    y = np.asarray((x,), dtype=np.float32)
    x2 = y * 0.5
    i = y.view(np.int32)
    i = np.right_shift(i, 1)
    i = 1597463007 - i
    y = i.view(np.float32)
    y = y * (1.5 - x2 * y * y)
    return float(y[0])


SCALING = _q_rsqrt(HD)

_CACHE = {}


STATS_SUB = 4                     # row-max from every-Nth score column
EXP_DVE_SLOTS = (1, 4, 7, 10, 13)  # of 16 mcp slots per head, exp on DVE
SHIFT_GPDMA = True                # kT/qT partition-shift DMAs via SWDGE


def _build():
    key = ("nc", STATS_SUB, EXP_DVE_SLOTS, SHIFT_GPDMA)
    if key in _CACHE:
        return _CACHE[key]

    import concourse.bass as bass
    import concourse.mybir as mybir
    import concourse.tile as tile
    from concourse import bacc
    from concourse.masks import make_identity

    f16 = mybir.dt.float16
    f32 = mybir.dt.float32
    i16 = mybir.dt.int16
    AF = mybir.ActivationFunctionType
    ALU = mybir.AluOpType

    # Schraudolph applied directly in the fp16 bit domain
    edt = i16
    expA = 1024.0 / float(np.log(2.0))
    expB = 15.0 * 1024.0 + (GIST_B / 8192.0 - 130048.0)

    nc = bacc.Bacc("TRN2", target_bir_lowering=False, debug=False,
                   num_devices=NCORES)

    d_xT = nc.dram_tensor("xT", (128, 4, SEQ), f16, kind="ExternalInput")
    d_xqT = nc.dram_tensor("xqT", (128, 4, LSP), f16, kind="ExternalInput")
    d_wq = nc.dram_tensor("wq", (128, 4, DIM), f16, kind="ExternalInput")
    d_wk = nc.dram_tensor("wk", (128, 4, DIM), f16, kind="ExternalInput")
    d_wv = nc.dram_tensor("wv", (128, 4, DIM), f16, kind="ExternalInput")
    d_wo = nc.dram_tensor("wo", (128, 4, DIM), f16, kind="ExternalInput")
    d_bqp = nc.dram_tensor("bqp", (128, 4), f32, kind="ExternalInput")
    d_bkp = nc.dram_tensor("bkp", (128, 4), f32, kind="ExternalInput")
    d_crow = nc.dram_tensor("crow", (DIM,), f32, kind="ExternalInput")
    d_ones = nc.dram_tensor("ones16", (H, SEQ), f16, kind="ExternalInput")
    d_y = nc.dram_tensor("y", (LSP, DIM), f32, kind="ExternalOutput")

    MSP = SEQ // 128  # 32 m chunks
    exp_ctr = [0]

    with tile.TileContext(nc) as tc:
        with (
            tc.tile_pool(name="const", bufs=1) as cp,
            tc.tile_pool(name="big", bufs=1) as bp,
            tc.tile_pool(name="tmp", bufs=6) as tp,
            tc.tile_pool(name="small", bufs=4) as sp,
            tc.tile_pool(name="t32p", bufs=4) as t32p,
            tc.tile_pool(name="ps", bufs=1, space="PSUM") as ps,
        ):
            # ---- constants / inputs in SBUF
            kT_aug = bp.tile([65, H, SEQ], f16)     # per-head k^T + ones row
            xT = cp.tile([128, 4, SEQ], f16)
            xqT = cp.tile([128, 4, LSP], f16)
            wq = cp.tile([128, 4, DIM], f16)
            wk = cp.tile([128, 4, DIM], f16)
            wv = cp.tile([128, 4, DIM], f16)
            wo = cp.tile([128, 4, DIM], f16)
            bqp = cp.tile([128, 4], f32)
            bkp = cp.tile([128, 4], f32)
            crow_b = cp.tile([128, DIM], f32)
            biasB = cp.tile([128, 1], f32)
            idf16 = cp.tile([128, 128], f16)
            negI = cp.tile([128, 128], f16)

            nc.sync.dma_start(out=wk, in_=d_wk[:, :, :])
            nc.sync.dma_start(out=bkp, in_=d_bkp[:, :])
            nc.sync.dma_start(out=wq, in_=d_wq[:, :, :])
            nc.sync.dma_start(out=bqp, in_=d_bqp[:, :])
            nc.sync.dma_start(out=xqT, in_=d_xqT[:, :, :])
            nc.sync.dma_start(out=kT_aug[64:65, :, :], in_=d_ones[:, :])
            for dc in range(4):
                nc.sync.dma_start(out=xT[:, dc, :], in_=d_xT[:, dc, :])
            nc.sync.dma_start(out=wv, in_=d_wv[:, :, :])
            nc.sync.dma_start(out=wo, in_=d_wo[:, :, :])
            crow_ap = d_crow[:]
            crow_bcast = bass.AP(tensor=crow_ap.tensor, offset=crow_ap.offset,
                                 ap=[[0, 128]] + list(crow_ap.ap))
            nc.sync.dma_start(out=crow_b, in_=crow_bcast)

            nc.vector.memset(biasB, expB)
            make_identity(nc, idf16)
            nc.scalar.mul(negI, idf16, -1.0)

            # ---- persistent activations
            qT_aug = bp.tile([65, H, LSP], f16)     # per-head q^T + (-mu) row
            v_sb = bp.tile([128, MSP, H, 65], f16)  # v + ones col, m-major
            attnT = bp.tile([128, 4, LSP], f16)
            nc.vector.memset(v_sb[:, :, :, 64:65], 1.0)

            shift_eng = nc.gpsimd if SHIFT_GPDMA else nc.sync

            # ---- projections
            def emit_qpairs():
                for pp in range(2):
                    qp = ps.tile([128, 1024], f32, tag="psB", name="qp")
                    for k in range(2):
                        p = 2 * pp + k
                        for dc in range(4):
                            nc.tensor.matmul(
                                qp[:, 512 * k:512 * k + 512],
                                wq[:, dc, 128 * p:128 * p + 128],
                                xqT[:, dc, :], start=(dc == 0), stop=(dc == 3))
                        tmp = tp.tile([128, 512], f16, tag="tmpq")
                        nc.scalar.activation(
                            out=tmp, in_=qp[:, 512 * k:512 * k + 512],
                            func=AF.Identity, bias=bqp[:, p:p + 1], scale=1.0)
                        shift_eng.dma_start(out=qT_aug[0:64, 2 * p, :],
                                            in_=tmp[0:64, :])
                        shift_eng.dma_start(out=qT_aug[0:64, 2 * p + 1, :],
                                            in_=tmp[64:128, :])

            def emit_k_startup():
                # jp=0 for all p: 8 tiles accumulate dc-interleaved so PE
                # consumes each arriving xT chunk instead of waiting for the
                # last one; borrows every pre-stats-idle psum bank.
                f8 = []
                slots = [ps.tile([128, 1024], f32, tag="psB", name="ila"),
                         ps.tile([128, 1024], f32, tag="psB", name="ilb"),
                         ps.tile([128, 1024], f32, tag="psB", name="ilc")]
                for p in range(4):
                    for k in range(2):
                        idx = 2 * p + k
                        if idx < 6:
                            t = slots[idx // 2][:, 512 * (idx % 2):
                                                512 * (idx % 2) + 512]
                        elif idx == 6:
                            t = ps.tile([128, 512], f32, tag="psM",
                                        name="ild")
                        else:
                            t = ps.tile([128, 512], f32, tag="psN",
                                        name="ile")
                        f8.append((t, p, k))
                for dc in range(4):
                    for t, p, k in f8:
                        nc.tensor.matmul(
                            t, wk[:, dc, 128 * p:128 * p + 128],
                            xT[:, dc, 512 * k:512 * k + 512],
                            start=(dc == 0), stop=(dc == 3))
                for p in range(4):
                    tmp = tp.tile([128, 1024], f16, tag="tmpk", name="tmpk")
                    for k in range(2):
                        t = f8[2 * p + k][0]
                        nc.scalar.activation(
                            out=tmp[:, 512 * k:512 * k + 512], in_=t,
                            func=AF.Identity, bias=bkp[:, p:p + 1], scale=1.0)
                    shift_eng.dma_start(out=kT_aug[0:64, 2 * p, 0:1024],
                                        in_=tmp[0:64, :])
                    shift_eng.dma_start(out=kT_aug[0:64, 2 * p + 1, 0:1024],
                                        in_=tmp[64:128, :])

            def emit_kpair(p, jp):
                tmp = tp.tile([128, 1024], f16, tag="tmpk", name="tmpk")
                kp = ps.tile([128, 1024], f32, tag="psB", name="kp")
                for k in range(2):
                    j = 2 * jp + k
                    for dc in range(4):
                        nc.tensor.matmul(
                            kp[:, 512 * k:512 * k + 512],
                            wk[:, dc, 128 * p:128 * p + 128],
                            xT[:, dc, 512 * j:512 * j + 512],
                            start=(dc == 0), stop=(dc == 3))
                    nc.scalar.activation(
                        out=tmp[:, 512 * k:512 * k + 512],
                        in_=kp[:, 512 * k:512 * k + 512],
                        func=AF.Identity, bias=bkp[:, p:p + 1], scale=1.0)
                sl = slice(1024 * jp, 1024 * jp + 1024)
                shift_eng.dma_start(out=kT_aug[0:64, 2 * p, sl],
                                    in_=tmp[0:64, :])
                shift_eng.dma_start(out=kT_aug[0:64, 2 * p + 1, sl],
                                    in_=tmp[64:128, :])

            # ---- stats: subsampled row max for one lc block of head h.
            # Split in two stages so the murow matmul (which waits on the
            # DVE reduce) never head-of-line blocks the in-order PE queue.
            def mk_stats1(h, lc, box):
                def go():
                    st = ps.tile([128, 1024], f32, tag="psB", name="st")
                    for k in range(2):
                        m0 = (SEQ // 2) * k
                        nc.tensor.matmul(
                            st[:, 512 * k:512 * k + 512],
                            qT_aug[0:64, h, 128 * lc:128 * lc + 128],
                            kT_aug[0:64, h, m0:m0 + SEQ // 2:STATS_SUB],
                            start=True, stop=True)
                    mucol = sp.tile([128, 1], f16, tag="mucol")
                    nc.vector.reduce_max(mucol, st,
                                         axis=mybir.AxisListType.X)
                    box.append(mucol)
                return go

            def mk_stats2(h, lc, box):
                def go():
                    mucol = box.pop(0)
                    murow = ps.tile([128, 128], f32, tag="psM", name="murow")
                    nc.tensor.matmul(murow[64:65, :], mucol, negI,
                                     start=True, stop=True)
                    nc.vector.tensor_copy(
                        out=qT_aug[64:65, h, 128 * lc:128 * lc + 128],
                        in_=murow[64:65, :])
                return go



            # ---- v projection (with stats 0/1 interleaved)
            def emit_vproj(sched):
                for mcp in range(16):
                    for fn in sched.get(mcp, ()):
                        fn()
                    vp = ps.tile([128, 1024], f32, tag="psB", name="vp")
                    for k in range(2):
                        mc = 2 * mcp + k
                        for dc in range(4):
                            nc.tensor.matmul(
                                vp[:, 512 * k:512 * k + 512],
                                xT[:, dc, 128 * mc:128 * mc + 128],
                                wv[:, dc, :], start=(dc == 0), stop=(dc == 3))
                        vsrc = vp[:, 512 * k:512 * k + 512].rearrange(
                            "p (h d) -> p h d", h=H)
                        nc.scalar.activation(out=v_sb[:, mc, :, 0:64],
                                             in_=vsrc, func=AF.Copy)

            # ---- per-head round: scores^T + fastexp + PV in [l, d]
            def emit_exp(dst, sT):
                c = exp_ctr[0]
                exp_ctr[0] += 1
                if c % 16 in EXP_DVE_SLOTS:
                    nc.vector.tensor_scalar(
                        out=dst, in0=sT, scalar1=expA, scalar2=expB,
                        op0=ALU.mult, op1=ALU.add)
                else:
                    nc.scalar.activation(out=dst, in_=sT, func=AF.Identity,
                                         bias=biasB, scale=expA)

            def emit_pv(item, onat, h, started):
                mcp, et = item
                esrc = et.bitcast(f16)
                for k in range(2):
                    mc = 2 * mcp + k
                    for lb in range(4):
                        nc.tensor.matmul(
                            onat[:, lb, 0:65],
                            esrc[:, 512 * k + 128 * lb:
                                 512 * k + 128 * lb + 128],
                            v_sb[:, mc, h, :],
                            start=not started[0], stop=(mc == MSP - 1),
                            skip_group_check=True)
                        started[0] = True

            def emit_round(h, sched):
                onat = ps.tile([128, 4, 128], f32, tag="psN", name="onat")
                started = [False]
                pvq = []
                for mcp in range(16):
                    for fn in sched.get(mcp, ()):
                        fn()
                    sTp = ps.tile([128, 1024], f32, tag="psB", name="sTp")
                    for k in range(2):
                        mc = 2 * mcp + k
                        nc.tensor.matmul(
                            sTp[:, 512 * k:512 * k + 512],
                            kT_aug[:, h, 128 * mc:128 * mc + 128],
                            qT_aug[:, h, :], start=True, stop=True)
                    et = t32p.tile([128, 1024], edt, tag="t32", name="et")
                    emit_exp(et, sTp)
                    pvq.append((mcp, et))
                    if len(pvq) >= 3:
                        emit_pv(pvq.pop(0), onat, h, started)
                while pvq:
                    emit_pv(pvq.pop(0), onat, h, started)
                return onat

            # ---- out stage for head h (normalize in [l, d], transpose back)
            def mk_norm(onat_h, h, anat):
                def go():
                    rcol = sp.tile([128, 4, 1], f32, tag="rcol")
                    nc.vector.reciprocal(rcol, onat_h[:, :, 64:65])
                    for lc in range(4):
                        nc.vector.tensor_scalar_mul(
                            anat[:, lc, :], onat_h[:, lc, 0:64],
                            rcol[:, lc, :])
                return go

            def mk_att(h, anat):
                def go():
                    aT4 = ps.tile([64, 512], f16, tag="psM", name="aT4")
                    for lc in range(4):
                        # transposes share one psum bank: single start mark
                        nc.tensor.matmul(
                            aT4[0:64, 128 * lc:128 * lc + 128],
                            anat[:, lc, :], idf16[:, 0:128],
                            is_transpose=True, start=(lc == 0), stop=True,
                            skip_group_check=True)
                    hb = 64 * (h % 2)
                    nc.vector.tensor_copy(out=attnT[hb:hb + 64, h // 2, :],
                                          in_=aT4)
                return go

            # ---- emit program
            emit_qpairs()
            emit_k_startup()
            for p in range(4):
                for jp in range(1, 4):
                    emit_kpair(p, jp)
            # stats(0)/stats(1) interleaved into the v-projection: stage1
            # every other mcp (psB slot each), stage2 trailing by 3 slots
            vsched = {}
            vbox = {0: [], 1: []}
            for i in range(2):
                for lc in range(4):
                    j = 4 * i + lc
                    vsched.setdefault(2 * j, []).append(
                        mk_stats1(i, lc, vbox[i]))
                    vsched.setdefault(min(2 * j + 3, 15), []).append(
                        mk_stats2(i, lc, vbox[i]))
            emit_vproj(vsched)

            prev = None
            for h in range(H):
                sched = {}
                box = []
                if h + 2 < H:
                    # 3 stats matmul-pairs at the boundary (ring is free),
                    # the 4th early in the round; murow stages trail inside
                    for lc in range(3):
                        mk_stats1(h + 2, lc, box)()
                    sched[2] = [mk_stats1(h + 2, 3, box)]
                    for j, lc in enumerate(range(4)):
                        sched.setdefault(4 + 2 * j, []).append(
                            mk_stats2(h + 2, lc, box))
                if prev is not None:
                    ph, ponat = prev
                    anat = sp.tile([128, 4, 64], f16, tag="anat",
                                   name="anat")
                    sched.setdefault(0, []).insert(0, mk_norm(ponat, ph,
                                                              anat))
                    sched.setdefault(1, []).append(mk_att(ph, anat))
                onat = emit_round(h, sched)
                prev = (h, onat)

            ph, ponat = prev
            anat = sp.tile([128, 4, 64], f16, tag="anat", name="anat")
            mk_norm(ponat, ph, anat)()
            mk_att(ph, anat)()

            # ---- output projection
            for lc in range(4):
                yp = ps.tile([128, 1024], f32, tag="psB", name="yp")
                for p in range(4):
                    nc.tensor.matmul(
                        yp[:, 0:512],
                        attnT[:, p, 128 * lc:128 * lc + 128],
                        wo[:, p, :], start=(p == 0), stop=(p == 3))
                y_sb = sp.tile([128, DIM], f32, tag="ysb")
                nc.vector.tensor_add(y_sb, yp[:, 0:512], crow_b)
                nc.sync.dma_start(out=d_y[128 * lc:128 * lc + 128, :],
                                  in_=y_sb)

    nc.compile()
    _CACHE[key] = nc
    return nc


def prep_in_maps(x, Wq, bq, Wk, bk, Wv, bv, Wout, tgt_len):
    assert int(tgt_len) == SEQ
    f16 = np.float16
    f32c = lambda a: np.asarray(a, dtype=np.float32)

    x, Wq, bq, Wk, bk = f32c(x), f32c(Wq), f32c(bq), f32c(Wk), f32c(bk)
    Wv, bv, Wout = f32c(Wv), f32c(bv), f32c(Wout)

    def chunk4(a, w):  # [512, w] -> [128, 4, w]
        return np.ascontiguousarray(
            a.reshape(4, 128, w).transpose(1, 0, 2))

    xT = np.ascontiguousarray(x.T)
    xT16 = chunk4(xT, SEQ).astype(f16)
    wq16 = chunk4(np.ascontiguousarray(Wq.T) * np.float32(SCALING),
                  DIM).astype(f16)
    wk16 = chunk4(np.ascontiguousarray(Wk.T), DIM).astype(f16)
    wv16 = chunk4(np.ascontiguousarray(Wv.T), DIM).astype(f16)
    wo16 = chunk4(np.ascontiguousarray(Wout.T), DIM).astype(f16)
    bqp = np.ascontiguousarray((bq * np.float32(SCALING)).reshape(4, 128).T)
    bkp = np.ascontiguousarray(bk.reshape(4, 128).T)
    crow = np.ascontiguousarray(Wout @ bv).astype(np.float32)
    ones16 = np.ones((H, SEQ), f16)

    in_maps = []
    for c in range(NCORES):
        xq16 = np.ascontiguousarray(xT16[:, :, LSP * c:LSP * (c + 1)])
        in_maps.append({
            "xT": xT16, "xqT": xq16, "wq": wq16, "wk": wk16, "wv": wv16,
            "wo": wo16, "bqp": bqp, "bkp": bkp, "crow": crow,
            "ones16": ones16,
        })
    return in_maps


def kernel(**inputs):
    from concourse.bass_utils import run_bass_kernel_spmd
    in_maps = prep_in_maps(**inputs)
    nc = _build()
    res = run_bass_kernel_spmd(nc, in_maps, core_ids=list(range(NCORES)))
    y = np.concatenate([r["y"] for r in res.results], axis=0)
    return y.astype(np.float32)


# revision 27
# speedup vs baseline: 1.0011x; 1.0011x over previous
"""Trainium2 Bass kernel for MultiHeadedSelfAttention (fastexp softmax).

Sharding: sequence-parallel over 8 cores. Each core computes K/V for the
full sequence and attention for its own 512-row query block; outputs are
disjoint row blocks of the final [4096, 512] result, so no collectives.

Device layout is "transposed everything": activations stored feature-major
(d on partitions) so projections and scores feed the PE contraction dim
directly. Softmax row-max is computed from a stride-STATS_SUB subsampled
[l, m] score pass (DVE reduce); the softmax ratio is invariant to a uniform
row shift, so a slightly-low max only perturbs the Schraudolph epsilon
pattern by ~eps' * delta. Scores are then recomputed transposed [m, l]
with the row max injected as an extra contraction row (ones x -mu), so the
exponent input arrives in PSUM already max-subtracted. The Schraudolph
fastexp is applied in the fp16 bit domain (ACT affine with int16 output,
or DVE tensor_scalar for a share of tiles to balance engines). PV runs in
[l, d] orientation (full 128-row PE utilization): four l-block
accumulators share one PSUM bank using a single start=True zero-region
mark. Row sums come free as a ones column appended to V; normalization
happens in [l, d] before a single f16 transpose back to attnT.
"""

import numpy as np

DIM = 512
H = 8
HD = 64
SEQ = 4096
NCORES = 8
LSP = SEQ // NCORES  # 512 query rows per core

GIST_A = 12102203.17133801
GIST_B = 1064986823.0


def _q_rsqrt(x):
    y = np.asarray((x,), dtype=np.float32)
    x2 = y * 0.5
    i = y.view(np.int32)
    i = np.right_shift(i, 1)
    i = 1597463007 - i
    y = i.view(np.float32)
    y = y * (1.5 - x2 * y * y)
    return float(y[0])


SCALING = _q_rsqrt(HD)

_CACHE = {}


STATS_SUB = 4                     # row-max from every-Nth score column
EXP_DVE_SLOTS = (1, 4, 7, 10, 13)  # of 16 mcp slots per head, exp on DVE
SHIFT_GPDMA = True                # kT/qT partition-shift DMAs via SWDGE


def _build():
    key = ("nc", STATS_SUB, EXP_DVE_SLOTS, SHIFT_GPDMA)
    if key in _CACHE:
        return _CACHE[key]

    import concourse.bass as bass
    import concourse.mybir as mybir
    import concourse.tile as tile
    from concourse import bacc
    from concourse.masks import make_identity

    f16 = mybir.dt.float16
    f32 = mybir.dt.float32
    i16 = mybir.dt.int16
    AF = mybir.ActivationFunctionType
    ALU = mybir.AluOpType

    # Schraudolph applied directly in the fp16 bit domain
    edt = i16
    expA = 1024.0 / float(np.log(2.0))
    expB = 15.0 * 1024.0 + (GIST_B / 8192.0 - 130048.0)

    nc = bacc.Bacc("TRN2", target_bir_lowering=False, debug=False,
                   num_devices=NCORES)

    d_xT = nc.dram_tensor("xT", (128, 4, SEQ), f16, kind="ExternalInput")
    d_xqT = nc.dram_tensor("xqT", (128, 4, LSP), f16, kind="ExternalInput")
    d_wq = nc.dram_tensor("wq", (128, 4, DIM), f16, kind="ExternalInput")
    d_wk = nc.dram_tensor("wk", (128, 4, DIM), f16, kind="ExternalInput")
    d_wv = nc.dram_tensor("wv", (128, 4, DIM), f16, kind="ExternalInput")
    d_wo = nc.dram_tensor("wo", (128, 4, DIM), f16, kind="ExternalInput")
    d_bqp = nc.dram_tensor("bqp", (128, 4), f32, kind="ExternalInput")
    d_bkp = nc.dram_tensor("bkp", (128, 4), f32, kind="ExternalInput")
    d_crow = nc.dram_tensor("crow", (DIM,), f32, kind="ExternalInput")
    d_ones = nc.dram_tensor("ones16", (H, SEQ), f16, kind="ExternalInput")
    d_y = nc.dram_tensor("y", (LSP, DIM), f32, kind="ExternalOutput")

    MSP = SEQ // 128  # 32 m chunks
    exp_ctr = [0]

    with tile.TileContext(nc) as tc:
        with (
            tc.tile_pool(name="const", bufs=1) as cp,
            tc.tile_pool(name="big", bufs=1) as bp,
            tc.tile_pool(name="tmp", bufs=6) as tp,
            tc.tile_pool(name="small", bufs=4) as sp,
            tc.tile_pool(name="t32p", bufs=6) as t32p,
            tc.tile_pool(name="ps", bufs=1, space="PSUM") as ps,
        ):
            # ---- constants / inputs in SBUF
            kT_aug = bp.tile([65, H, SEQ], f16)     # per-head k^T + ones row
            xT = cp.tile([128, 4, SEQ], f16)
            xqT = cp.tile([128, 4, LSP], f16)
            wq = cp.tile([128, 4, DIM], f16)
            wk = cp.tile([128, 4, DIM], f16)
            wv = cp.tile([128, 4, DIM], f16)
            wo = cp.tile([128, 4, DIM], f16)
            bqp = cp.tile([128, 4], f32)
            bkp = cp.tile([128, 4], f32)
            crow_b = cp.tile([128, DIM], f32)
            biasB = cp.tile([128, 1], f32)
            idf16 = cp.tile([128, 128], f16)
            negI = cp.tile([128, 128], f16)

            nc.sync.dma_start(out=wk, in_=d_wk[:, :, :])
            nc.sync.dma_start(out=bkp, in_=d_bkp[:, :])
            nc.sync.dma_start(out=wq, in_=d_wq[:, :, :])
            nc.sync.dma_start(out=bqp, in_=d_bqp[:, :])
            nc.sync.dma_start(out=xqT, in_=d_xqT[:, :, :])
            nc.sync.dma_start(out=kT_aug[64:65, :, :], in_=d_ones[:, :])
            for dc in range(4):
                nc.sync.dma_start(out=xT[:, dc, :], in_=d_xT[:, dc, :])
            nc.sync.dma_start(out=wv, in_=d_wv[:, :, :])
            nc.sync.dma_start(out=wo, in_=d_wo[:, :, :])
            crow_ap = d_crow[:]
            crow_bcast = bass.AP(tensor=crow_ap.tensor, offset=crow_ap.offset,
                                 ap=[[0, 128]] + list(crow_ap.ap))
            nc.sync.dma_start(out=crow_b, in_=crow_bcast)

            nc.vector.memset(biasB, expB)
            make_identity(nc, idf16)
            nc.scalar.mul(negI, idf16, -1.0)

            # ---- persistent activations
            qT_aug = bp.tile([65, H, LSP], f16)     # per-head q^T + (-mu) row
            v_sb = bp.tile([128, MSP, H, 65], f16)  # v + ones col, m-major
            attnT = bp.tile([128, 4, LSP], f16)
            nc.vector.memset(v_sb[:, :, :, 64:65], 1.0)

            shift_eng = nc.gpsimd if SHIFT_GPDMA else nc.sync

            # ---- projections
            def emit_qpairs():
                for pp in range(2):
                    qp = ps.tile([128, 1024], f32, tag="psB", name="qp")
                    for k in range(2):
                        p = 2 * pp + k
                        for dc in range(4):
                            nc.tensor.matmul(
                                qp[:, 512 * k:512 * k + 512],
                                wq[:, dc, 128 * p:128 * p + 128],
                                xqT[:, dc, :], start=(dc == 0), stop=(dc == 3))
                        tmp = tp.tile([128, 512], f16, tag="tmpq")
                        nc.scalar.activation(
                            out=tmp, in_=qp[:, 512 * k:512 * k + 512],
                            func=AF.Identity, bias=bqp[:, p:p + 1], scale=1.0)
                        shift_eng.dma_start(out=qT_aug[0:64, 2 * p, :],
                                            in_=tmp[0:64, :])
                        shift_eng.dma_start(out=qT_aug[0:64, 2 * p + 1, :],
                                            in_=tmp[64:128, :])

            def emit_k_startup():
                # jp=0 for all p: 8 tiles accumulate dc-interleaved so PE
                # consumes each arriving xT chunk instead of waiting for the
                # last one; borrows every pre-stats-idle psum bank.
                f8 = []
                slots = [ps.tile([128, 1024], f32, tag="psB", name="ila"),
                         ps.tile([128, 1024], f32, tag="psB", name="ilb"),
                         ps.tile([128, 1024], f32, tag="psB", name="ilc")]
                for p in range(4):
                    for k in range(2):
                        idx = 2 * p + k
                        if idx < 6:
                            t = slots[idx // 2][:, 512 * (idx % 2):
                                                512 * (idx % 2) + 512]
                        elif idx == 6:
                            t = ps.tile([128, 512], f32, tag="psM",
                                        name="ild")
                        else:
                            t = ps.tile([128, 512], f32, tag="psN",
                                        name="ile")
                        f8.append((t, p, k))
                for dc in range(4):
                    for t, p, k in f8:
                        nc.tensor.matmul(
                            t, wk[:, dc, 128 * p:128 * p + 128],
                            xT[:, dc, 512 * k:512 * k + 512],
                            start=(dc == 0), stop=(dc == 3))
                for p in range(4):
                    tmp = tp.tile([128, 1024], f16, tag="tmpk", name="tmpk")
                    for k in range(2):
                        t = f8[2 * p + k][0]
                        nc.scalar.activation(
                            out=tmp[:, 512 * k:512 * k + 512], in_=t,
                            func=AF.Identity, bias=bkp[:, p:p + 1], scale=1.0)
                    shift_eng.dma_start(out=kT_aug[0:64, 2 * p, 0:1024],
                                        in_=tmp[0:64, :])
                    shift_eng.dma_start(out=kT_aug[0:64, 2 * p + 1, 0:1024],
                                        in_=tmp[64:128, :])

            def emit_kpair(p, jp):
                tmp = tp.tile([128, 1024], f16, tag="tmpk", name="tmpk")
                kp = ps.tile([128, 1024], f32, tag="psB", name="kp")
                for k in range(2):
                    j = 2 * jp + k
                    for dc in range(4):
                        nc.tensor.matmul(
                            kp[:, 512 * k:512 * k + 512],
                            wk[:, dc, 128 * p:128 * p + 128],
                            xT[:, dc, 512 * j:512 * j + 512],
                            start=(dc == 0), stop=(dc == 3))
                    nc.scalar.activation(
                        out=tmp[:, 512 * k:512 * k + 512],
                        in_=kp[:, 512 * k:512 * k + 512],
                        func=AF.Identity, bias=bkp[:, p:p + 1], scale=1.0)
                sl = slice(1024 * jp, 1024 * jp + 1024)
                shift_eng.dma_start(out=kT_aug[0:64, 2 * p, sl],
                                    in_=tmp[0:64, :])
                shift_eng.dma_start(out=kT_aug[0:64, 2 * p + 1, sl],
                                    in_=tmp[64:128, :])

            # ---- stats: subsampled row max for one lc block of head h.
            # Split in two stages so the murow matmul (which waits on the
            # DVE reduce) never head-of-line blocks the in-order PE queue.
            def mk_stats1(h, lc, box):
                def go():
                    st = ps.tile([128, 1024], f32, tag="psB", name="st")
                    for k in range(2):
                        m0 = (SEQ // 2) * k
                        nc.tensor.matmul(
                            st[:, 512 * k:512 * k + 512],
                            qT_aug[0:64, h, 128 * lc:128 * lc + 128],
                            kT_aug[0:64, h, m0:m0 + SEQ // 2:STATS_SUB],
                            start=True, stop=True)
                    mucol = sp.tile([128, 1], f16, tag="mucol")
                    nc.vector.reduce_max(mucol, st,
                                         axis=mybir.AxisListType.X)
                    box.append(mucol)
                return go

            def mk_stats2(h, lc, box):
                def go():
                    mucol = box.pop(0)
                    murow = ps.tile([128, 128], f32, tag="psM", name="murow")
                    nc.tensor.matmul(murow[64:65, :], mucol, negI,
                                     start=True, stop=True)
                    nc.vector.tensor_copy(
                        out=qT_aug[64:65, h, 128 * lc:128 * lc + 128],
                        in_=murow[64:65, :])
                return go



            # ---- v projection (with stats 0/1 interleaved)
            def emit_vproj(sched):
                for mcp in range(16):
                    for fn in sched.get(mcp, ()):
                        fn()
                    vp = ps.tile([128, 1024], f32, tag="psB", name="vp")
                    for k in range(2):
                        mc = 2 * mcp + k
                        for dc in range(4):
                            nc.tensor.matmul(
                                vp[:, 512 * k:512 * k + 512],
                                xT[:, dc, 128 * mc:128 * mc + 128],
                                wv[:, dc, :], start=(dc == 0), stop=(dc == 3))
                        vsrc = vp[:, 512 * k:512 * k + 512].rearrange(
                            "p (h d) -> p h d", h=H)
                        nc.scalar.activation(out=v_sb[:, mc, :, 0:64],
                                             in_=vsrc, func=AF.Copy)

            # ---- per-head round: scores^T + fastexp + PV in [l, d]
            def emit_exp(dst, sT):
                c = exp_ctr[0]
                exp_ctr[0] += 1
                if c % 16 in EXP_DVE_SLOTS:
                    nc.vector.tensor_scalar(
                        out=dst, in0=sT, scalar1=expA, scalar2=expB,
                        op0=ALU.mult, op1=ALU.add)
                else:
                    nc.scalar.activation(out=dst, in_=sT, func=AF.Identity,
                                         bias=biasB, scale=expA)

            def emit_pv(item, onat, h, started):
                mcp, et = item
                esrc = et.bitcast(f16)
                for k in range(2):
                    mc = 2 * mcp + k
                    for lb in range(4):
                        nc.tensor.matmul(
                            onat[:, lb, 0:65],
                            esrc[:, 512 * k + 128 * lb:
                                 512 * k + 128 * lb + 128],
                            v_sb[:, mc, h, :],
                            start=not started[0], stop=(mc == MSP - 1),
                            skip_group_check=True)
                        started[0] = True

            def emit_round(h, sched):
                onat = ps.tile([128, 4, 128], f32, tag="psN", name="onat")
                started = [False]
                pvq = []
                for mcp in range(16):
                    for fn in sched.get(mcp, ()):
                        fn()
                    sTp = ps.tile([128, 1024], f32, tag="psB", name="sTp")
                    for k in range(2):
                        mc = 2 * mcp + k
                        nc.tensor.matmul(
                            sTp[:, 512 * k:512 * k + 512],
                            kT_aug[:, h, 128 * mc:128 * mc + 128],
                            qT_aug[:, h, :], start=True, stop=True)
                    et = t32p.tile([128, 1024], edt, tag="t32", name="et")
                    emit_exp(et, sTp)
                    pvq.append((mcp, et))
                    if len(pvq) >= 4:
                        emit_pv(pvq.pop(0), onat, h, started)
                while pvq:
                    emit_pv(pvq.pop(0), onat, h, started)
                return onat

            # ---- out stage for head h (normalize in [l, d], transpose back)
            def mk_norm(onat_h, h, anat):
                def go():
                    rcol = sp.tile([128, 4, 1], f32, tag="rcol")
                    nc.vector.reciprocal(rcol, onat_h[:, :, 64:65])
                    for lc in range(4):
                        nc.vector.tensor_scalar_mul(
                            anat[:, lc, :], onat_h[:, lc, 0:64],
                            rcol[:, lc, :])
                return go

            def mk_att(h, anat):
                def go():
                    aT4 = ps.tile([64, 512], f16, tag="psM", name="aT4")
                    for lc in range(4):
                        # transposes share one psum bank: single start mark
                        nc.tensor.matmul(
                            aT4[0:64, 128 * lc:128 * lc + 128],
                            anat[:, lc, :], idf16[:, 0:128],
                            is_transpose=True, start=(lc == 0), stop=True,
                            skip_group_check=True)
                    hb = 64 * (h % 2)
                    nc.vector.tensor_copy(out=attnT[hb:hb + 64, h // 2, :],
                                          in_=aT4)
                return go

            # ---- emit program
            emit_qpairs()
            emit_k_startup()
            for p in range(4):
                for jp in range(1, 4):
                    emit_kpair(p, jp)
            # stats(0)/stats(1) interleaved into the v-projection: stage1
            # every other mcp (psB slot each), stage2 trailing by 3 slots
            vsched = {}
            vbox = {0: [], 1: []}
            for i in range(2):
                for lc in range(4):
                    j = 4 * i + lc
                    vsched.setdefault(2 * j, []).append(
                        mk_stats1(i, lc, vbox[i]))
                    vsched.setdefault(min(2 * j + 3, 15), []).append(
                        mk_stats2(i, lc, vbox[i]))
            emit_vproj(vsched)

            prev = None
            for h in range(H):
                sched = {}
                box = []
                if h + 2 < H:
                    # 3 stats matmul-pairs at the boundary (ring is free),
                    # the 4th early in the round; murow stages trail inside
                    for lc in range(3):
                        mk_stats1(h + 2, lc, box)()
                    sched[2] = [mk_stats1(h + 2, 3, box)]
                    for j, lc in enumerate(range(4)):
                        sched.setdefault(4 + 2 * j, []).append(
                            mk_stats2(h + 2, lc, box))
                if prev is not None:
                    ph, ponat = prev
                    anat = sp.tile([128, 4, 64], f16, tag="anat",
                                   name="anat")
                    sched.setdefault(0, []).insert(0, mk_norm(ponat, ph,
                                                              anat))
                    sched.setdefault(1, []).append(mk_att(ph, anat))
                onat = emit_round(h, sched)
                prev = (h, onat)

            ph, ponat = prev
            anat = sp.tile([128, 4, 64], f16, tag="anat", name="anat")
            mk_norm(ponat, ph, anat)()
            mk_att(ph, anat)()

            # ---- output projection
            for lc in range(4):
                yp = ps.tile([128, 1024], f32, tag="psB", name="yp")
                for p in range(4):
                    nc.tensor.matmul(
                        yp[:, 0:512],
                        attnT[:, p, 128 * lc:128 * lc + 128],
                        wo[:, p, :], start=(p == 0), stop=(p == 3))
                y_sb = sp.tile([128, DIM], f32, tag="ysb")
                nc.vector.tensor_add(y_sb, yp[:, 0:512], crow_b)
                nc.sync.dma_start(out=d_y[128 * lc:128 * lc + 128, :],
                                  in_=y_sb)

    nc.compile()
    _CACHE[key] = nc
    return nc


def prep_in_maps(x, Wq, bq, Wk, bk, Wv, bv, Wout, tgt_len):
    assert int(tgt_len) == SEQ
    f16 = np.float16
    f32c = lambda a: np.asarray(a, dtype=np.float32)

    x, Wq, bq, Wk, bk = f32c(x), f32c(Wq), f32c(bq), f32c(Wk), f32c(bk)
    Wv, bv, Wout = f32c(Wv), f32c(bv), f32c(Wout)

    def chunk4(a, w):  # [512, w] -> [128, 4, w]
        return np.ascontiguousarray(
            a.reshape(4, 128, w).transpose(1, 0, 2))

    xT = np.ascontiguousarray(x.T)
    xT16 = chunk4(xT, SEQ).astype(f16)
    wq16 = chunk4(np.ascontiguousarray(Wq.T) * np.float32(SCALING),
                  DIM).astype(f16)
    wk16 = chunk4(np.ascontiguousarray(Wk.T), DIM).astype(f16)
    wv16 = chunk4(np.ascontiguousarray(Wv.T), DIM).astype(f16)
    wo16 = chunk4(np.ascontiguousarray(Wout.T), DIM).astype(f16)
    bqp = np.ascontiguousarray((bq * np.float32(SCALING)).reshape(4, 128).T)
    bkp = np.ascontiguousarray(bk.reshape(4, 128).T)
    crow = np.ascontiguousarray(Wout @ bv).astype(np.float32)
    ones16 = np.ones((H, SEQ), f16)

    in_maps = []
    for c in range(NCORES):
        xq16 = np.ascontiguousarray(xT16[:, :, LSP * c:LSP * (c + 1)])
        in_maps.append({
            "xT": xT16, "xqT": xq16, "wq": wq16, "wk": wk16, "wv": wv16,
            "wo": wo16, "bqp": bqp, "bkp": bkp, "crow": crow,
            "ones16": ones16,
        })
    return in_maps


def kernel(**inputs):
    from concourse.bass_utils import run_bass_kernel_spmd
    in_maps = prep_in_maps(**inputs)
    nc = _build()
    res = run_bass_kernel_spmd(nc, in_maps, core_ids=list(range(NCORES)))
    y = np.concatenate([r["y"] for r in res.results], axis=0)
    return y.astype(np.float32)
